# revision 9
# baseline (speedup 1.0000x reference)
"""BiMamba block Trainium2 kernel — 8 NeuronCores.

Sharding: core = (direction, batch, d_inner-half):  c = 4*dir + 2*b + half.
Each core runs the full sequence (L=1024) for one direction of one batch
element over half of d_inner (1024 channels), so the selective scan has no
cross-core sequential dependency.  Cross-core sums use three small
collectives: AllReduce over (half) pairs for the x_proj partials, an
AllReduce-pair + AllGather across directions for the out_proj partials
(the backward direction's contribution is time-reversed on-chip while
combining), and an AllReduce over the batch quad for the FFN partials
(d_ff is sharded 4-way across the quad).

Layout on chip is feature-major: activations live as [feature partitions,
time free].  The scan runs on the Vector engine's tensor_tensor_scan with
free dim ordered (state n major, time minor); dA = exp(A*delta) comes from
the Scalar engine with the A column as the per-partition activation scale.
"""

import numpy as np
import ml_dtypes

import concourse.bass as bass
import concourse.bacc as bacc
import concourse.mybir as mybir
import concourse.tile as tile
from contextlib import ExitStack

F32 = mybir.dt.float32
F16 = mybir.dt.float16
BF16 = mybir.dt.bfloat16
Alu = mybir.AluOpType
Act = mybir.ActivationFunctionType

P = 128
DM = 1024          # d_model
L = 1024           # sequence length
DIH = 1024         # d_inner half (per core)
NST = 16           # d_state
DTR = 64           # dt_rank
KT = DM // P       # 8  k-tiles over d_model
DT = DIH // P      # 8  d-tiles over d_inner-half
TSUB = 256         # scan sub-chunk length
NSUB = L // TSUB   # 4
DFQ = 1024         # d_ff quarter (per core)
EPS = 1e-5

_BF = ml_dtypes.bfloat16

PAIRS = [[0, 1], [2, 3], [4, 5], [6, 7]]
XDIR = [[0, 4], [1, 5], [2, 6], [3, 7]]
QUADS = [[0, 1, 4, 5], [2, 3, 6, 7]]


def _ln_stats(nc, pools, get_x, ones_col, rstd_sb, mean_sb):
    """Per-token mean/rstd over KT tiles of [128, L] f32 (features on
    partitions).  get_x(kt) returns the kt-th SBUF tile.

    Writes mean_sb [128, L] and rstd_sb [128, L] (broadcast to all
    partitions via gpsimd.partition_broadcast).
    """
    psum, scratch, statp = pools["psum"], pools["scratch"], pools["statp"]
    ps_m = [psum.tile([1, 512], F32, tag="mm", name=f"ps_m{_h}")
            for _h in range(2)]
    ps_s = [psum.tile([1, 512], F32, tag="mm", name=f"ps_s{_h}")
            for _h in range(2)]
    for kt in range(KT):
        xkt = get_x(kt)
        sq = scratch.tile([P, L], F32, tag="big", name="sq")
        nc.scalar.square(sq[:], xkt)
        for h in range(2):
            sl = slice(h * 512, (h + 1) * 512)
            nc.tensor.matmul(ps_m[h][:], ones_col[:], xkt[:, sl],
                             start=(kt == 0), stop=(kt == KT - 1))
            nc.tensor.matmul(ps_s[h][:], ones_col[:], sq[:, sl],
                             start=(kt == 0), stop=(kt == KT - 1))
    mean1 = statp.tile([1, L], F32, tag="stat", name="mean1")
    var1 = statp.tile([1, L], F32, tag="stat", name="var1")
    m2 = statp.tile([1, L], F32, tag="stat", name="m2")
    for h in range(2):
        sl = slice(h * 512, (h + 1) * 512)
        nc.vector.tensor_scalar_mul(mean1[:, sl], ps_m[h][:], 1.0 / DM)
        nc.vector.tensor_scalar_mul(var1[:, sl], ps_s[h][:], 1.0 / DM)
    nc.vector.tensor_tensor(m2[:], mean1[:], mean1[:], Alu.mult)
    nc.vector.tensor_tensor(var1[:], var1[:], m2[:], Alu.subtract)
    nc.vector.tensor_scalar_add(var1[:], var1[:], EPS)
    nc.scalar.sqrt(var1[:], var1[:])
    nc.vector.reciprocal(m2[:], var1[:])
    nc.gpsimd.partition_broadcast(mean_sb[:], mean1[:])
    nc.gpsimd.partition_broadcast(rstd_sb[:], m2[:])


def _ln_apply(nc, pools, get_x, mean_sb, rstd_sb, g_sb, b_sb, out_sb):
    """out = ((x - mean) * rstd) * g + b, per kt tile; out_sb bf16."""
    scratch = pools["scratch"]
    for kt in range(KT):
        xkt = get_x(kt)
        t1 = scratch.tile([P, L], F32, tag="big", name="t1")
        nc.vector.tensor_tensor(t1[:], xkt, mean_sb[:], Alu.subtract)
        nc.vector.tensor_tensor(t1[:], t1[:], rstd_sb[:], Alu.mult)
        nc.vector.tensor_scalar(out_sb[:, kt, :], t1[:],
                                g_sb[:, kt:kt + 1], b_sb[:, kt:kt + 1],
                                Alu.mult, Alu.add)


def build_program():
    nc = bacc.Bacc("TRN2", target_bir_lowering=False, debug=False,
                   num_devices=8)

    # ---- I/O ----
    xT_ln = nc.dram_tensor("xT_ln", [DM, L], F32, kind="ExternalInput")
    xT_res = nc.dram_tensor("xT_res", [DM, L], F32, kind="ExternalInput")
    ln1_g = nc.dram_tensor("ln1_g", [DM], F32, kind="ExternalInput")
    ln1_b = nc.dram_tensor("ln1_b", [DM], F32, kind="ExternalInput")
    ln2_g = nc.dram_tensor("ln2_g", [DM], F32, kind="ExternalInput")
    ln2_b = nc.dram_tensor("ln2_b", [DM], F32, kind="ExternalInput")
    win_t = nc.dram_tensor("win_t", [DM, 2 * DIH], BF16, kind="ExternalInput")
    conv_w = nc.dram_tensor("conv_w", [DIH, 4], F32, kind="ExternalInput")
    conv_b = nc.dram_tensor("conv_b", [DIH], F32, kind="ExternalInput")
    a_mat = nc.dram_tensor("a_mat", [DIH, NST], F32, kind="ExternalInput")
    xpw_t = nc.dram_tensor("xpw_t", [DIH, 96], BF16, kind="ExternalInput")
    dtw_t = nc.dram_tensor("dtw_t", [DTR, DIH], BF16, kind="ExternalInput")
    dt_b = nc.dram_tensor("dt_b", [DIH], F32, kind="ExternalInput")
    d_par = nc.dram_tensor("d_par", [DIH], F32, kind="ExternalInput")
    outw_t = nc.dram_tensor("outw_t", [DIH, DM], BF16, kind="ExternalInput")
    w1_t = nc.dram_tensor("w1_t", [DM, DFQ], BF16, kind="ExternalInput")
    b1_q = nc.dram_tensor("b1_q", [DFQ], F32, kind="ExternalInput")
    w2_t = nc.dram_tensor("w2_t", [DFQ, DM], BF16, kind="ExternalInput")
    b2_e = nc.dram_tensor("b2_e", [DM], F32, kind="ExternalInput")
    outT = nc.dram_tensor("outT", [DM, L], F32, kind="ExternalOutput")

    def vec_pt(dram_vec, pool, dt_, tag):
        """Load a [D] dram vector as [128, D//128] (col o = chans o*128..)."""
        t = pool.tile([P, dram_vec.shape[0] // P], dt_, tag=tag, name=tag)
        nc.sync.dma_start(t[:], dram_vec.rearrange("(o p) -> p o", p=P))
        return t

    with tile.TileContext(nc) as tc, ExitStack() as es:
        pc = es.enter_context(tc.tile_pool(name="const", bufs=1))
        psum = es.enter_context(tc.tile_pool(name="psum", bufs=6, space="PSUM"))
        scratch = es.enter_context(tc.tile_pool(name="scratch", bufs=3))
        statp = es.enter_context(tc.tile_pool(name="statp", bufs=3))
        dram = es.enter_context(tc.tile_pool(name="dram", bufs=1, space="DRAM"))
        pools = {"psum": psum, "scratch": scratch, "statp": statp}

        # constants
        ones_col = pc.tile([P, 1], F32, tag="ones")
        nc.vector.memset(ones_col[:], 1.0)
        g1 = vec_pt(ln1_g, pc, F32, "g1")
        b1v = vec_pt(ln1_b, pc, F32, "b1v")
        g2 = vec_pt(ln2_g, pc, F32, "g2")
        b2v = vec_pt(ln2_b, pc, F32, "b2v")
        cw = pc.tile([P, DT, 4], F32, tag="cw")
        nc.sync.dma_start(cw[:], conv_w.rearrange("(o p) k -> p o k", p=P))
        cb = vec_pt(conv_b, pc, F32, "cb")
        a_sb = pc.tile([P, DT, NST], F32, tag="a")
        nc.sync.dma_start(a_sb[:], a_mat.rearrange("(o p) n -> p o n", p=P))
        dtb = vec_pt(dt_b, pc, F32, "dtb")
        dpv = vec_pt(d_par, pc, F32, "dpv")
        b1s = vec_pt(b1_q, pc, F32, "b1s")
        b2s = vec_pt(b2_e, pc, F32, "b2s")

        bc_dram = dram.tile([2 * NST, L], BF16)

        with tc.tile_pool(name="pD", bufs=1) as pD:
            y_g = pD.tile([P, DT, L], BF16, tag="yg")
            outw = pD.tile([P, DT, DM], BF16, tag="outw")

            with tc.tile_pool(name="pact", bufs=1) as pact:
                # persistent activations for the mamba branch
                delta = pact.tile([P, DT, L], F16, tag="delta")
                u_bf = pact.tile([P, DT, L], BF16, tag="u")
                silz = pact.tile([P, DT, L], BF16, tag="silz")

                # ---------- Phase 0: LN1 ----------
                with tc.tile_pool(name="pA", bufs=1) as pA, \
                     tc.tile_pool(name="pAx", bufs=3) as pAx:
                    xlnv = xT_ln.rearrange("(o p) t -> p o t", p=P)

                    def _load_x1(kt):
                        xk = pAx.tile([P, L], F32, tag="xk", name="xk")
                        nc.sync.dma_start(xk[:], xlnv[:, kt, :])
                        return xk[:]

                    mean_sb = pA.tile([P, L], F32, tag="meanr")
                    rstd_sb = pA.tile([P, L], F32, tag="rstdr")
                    _ln_stats(nc, pools, _load_x1, ones_col, rstd_sb, mean_sb)
                    xnorm = pA.tile([P, KT, L], BF16, tag="xnorm")
                    _ln_apply(nc, pools, _load_x1, mean_sb, rstd_sb, g1, b1v,
                              xnorm)

                    # ---------- Phase 1: in_proj ----------
                    with tc.tile_pool(name="pW1", bufs=1) as pW1, \
                         tc.tile_pool(name="pB", bufs=1) as pB:
                        win = pW1.tile([P, KT, 2 * DIH], BF16, tag="win")
                        nc.sync.dma_start(
                            win[:], win_t.rearrange("(o p) e -> p o e", p=P))
                        xiT = pB.tile([P, DT, L + 3], BF16, tag="xi")
                        nc.vector.memset(xiT[:, :, 0:3], 0.0)
                        for m in range(2 * DT):
                            pss = [psum.tile([P, 512], F32, tag="mm",
                                             name="pss%d" % _h)
                                   for _h in range(2)]
                            for kt in range(KT):
                                for h in range(2):
                                    nc.tensor.matmul(
                                        pss[h][:],
                                        win[:, kt, m * P:(m + 1) * P],
                                        xnorm[:, kt, h * 512:(h + 1) * 512],
                                        start=(kt == 0), stop=(kt == KT - 1))
                            for h in range(2):
                                if m < DT:
                                    nc.scalar.copy(
                                        xiT[:, m,
                                            3 + h * 512:3 + (h + 1) * 512],
                                        pss[h][:])
                                else:
                                    nc.scalar.activation(
                                        silz[:, m - DT,
                                             h * 512:(h + 1) * 512],
                                        pss[h][:], Act.Silu)

                        # ---------- Phase 2: conv + silu ----------
                        for d in range(DT):
                            acc = scratch.tile([P, L], F32, tag="big",
                                               name="acc")
                            nc.vector.tensor_scalar(
                                acc[:], xiT[:, d, 0:L], cw[:, d, 0:1],
                                cb[:, d:d + 1], Alu.mult, Alu.add)
                            for k in (1, 2, 3):
                                nc.vector.scalar_tensor_tensor(
                                    acc[:], xiT[:, d, k:k + L],
                                    cw[:, d, k:k + 1], acc[:],
                                    Alu.mult, Alu.add)
                            nc.scalar.activation(u_bf[:, d, :], acc[:],
                                                 Act.Silu)

                        # ------- Phase 3: x_proj (+ pair AllReduce) -------
                        xpw = pB.tile([P, DT, 96], BF16, tag="xpw")
                        nc.sync.dma_start(
                            xpw[:], xpw_t.rearrange("(o p) r -> p o r", p=P))
                        psx = [psum.tile([96, 512], F32, tag="mm",
                                         name="psx%d" % _h) for _h in range(2)]
                        for d in range(DT):
                            for h in range(2):
                                nc.tensor.matmul(
                                    psx[h][:], xpw[:, d, :],
                                    u_bf[:, d, h * 512:(h + 1) * 512],
                                    start=(d == 0), stop=(d == DT - 1))
                        xd_in = dram.tile([96, L], F32)
                        xd_out = dram.tile([96, L], F32)
                        xdp = scratch.tile([P, L], F32, tag="big", name="xdp")
                        for h in range(2):
                            nc.vector.tensor_copy(
                                xdp[0:96, h * 512:(h + 1) * 512], psx[h][:])
                        nc.sync.dma_start(xd_in[:], xdp[0:96, :])
                        nc.gpsimd.collective_compute(
                            "AllReduce", Alu.add, replica_groups=PAIRS,
                            ins=[xd_in.opt()], outs=[xd_out.opt()])
                        xdbl = pB.tile([96, L], F32, tag="xdbl")
                        nc.sync.dma_start(xdbl[:], xd_out[:])

                        # ---------- Phase 4: dt proj + softplus ----------
                        dtw = pB.tile([DTR, DIH], BF16, tag="dtw")
                        nc.sync.dma_start(dtw[:], dtw_t[:])
                        dtraw = pB.tile([DTR, L], BF16, tag="dtraw")
                        nc.vector.tensor_copy(dtraw[:], xdbl[0:DTR, :])
                        for m in range(DT):
                            psd = [psum.tile([P, 512], F32, tag="mm",
                                             name="psd%d" % _h)
                                   for _h in range(2)]
                            for h in range(2):
                                nc.tensor.matmul(
                                    psd[h][:], dtw[:, m * P:(m + 1) * P],
                                    dtraw[:, h * 512:(h + 1) * 512],
                                    start=True, stop=True)
                                spe = scratch.tile([P, 512], F32,
                                                   tag="sp", name="spe")
                                nc.scalar.activation(spe[:], psd[h][:],
                                                     Act.Exp,
                                                     bias=dtb[:, m:m + 1])
                                nc.scalar.activation(
                                    delta[:, m, h * 512:(h + 1) * 512],
                                    spe[:], Act.Ln, bias=1.0)

                        # ------- Phase 5: B/C rows to DRAM (bf16) -------
                        bc_bf = pB.tile([2 * NST, L], BF16, tag="bc")
                        nc.vector.tensor_copy(bc_bf[:], xdbl[DTR:96, :])
                        nc.sync.dma_start(bc_dram[:], bc_bf[:])

                # prefetch out_proj weight (DMA overlaps the scan)
                nc.sync.dma_start(
                    outw[:], outw_t.rearrange("(o p) e -> p o e", p=P))

                # ---------- Phase 6: selective scan ----------
                with tc.tile_pool(name="pC", bufs=1) as pC, \
                     tc.tile_pool(name="pC2", bufs=2) as pC2:
                    hlast = pC.tile([P, DT, NST], BF16, tag="hlast")
                    for s in range(NSUB):
                        tsl = slice(s * TSUB, (s + 1) * TSUB)
                        b_sub = pC2.tile([P, NST, TSUB], BF16, tag="bsub",
                                         name="b_sub")
                        c_sub = pC2.tile([P, NST, TSUB], BF16, tag="csub",
                                         name="c_sub")
                        for n in range(NST):
                            nc.sync.dma_start(
                                b_sub[:, n, :],
                                bc_dram[n:n + 1, tsl].to_broadcast((P, TSUB)))
                            nc.sync.dma_start(
                                c_sub[:, n, :],
                                bc_dram[NST + n:NST + n + 1,
                                        tsl].to_broadcast((P, TSUB)))
                        for d in range(DT):
                            dA = pC2.tile([P, NST, TSUB], F32, tag="dA",
                                          name="dA")
                            for n in range(NST):
                                nc.scalar.activation(
                                    dA[:, n, :], delta[:, d, tsl], Act.Exp,
                                    scale=a_sb[:, d, n:n + 1])
                            du = pC.tile([P, TSUB], BF16, tag="du", name="du")
                            nc.vector.tensor_tensor(du[:], delta[:, d, tsl],
                                                    u_bf[:, d, tsl], Alu.mult)
                            dBu = pC.tile([P, NST, TSUB], BF16, tag="dBu",
                                          name="dBu")
                            nc.vector.tensor_tensor(
                                dBu[:],
                                du[:, None, :].to_broadcast((P, NST, TSUB)),
                                b_sub[:], Alu.mult)
                            # carry state across sub-chunks: fold it into the
                            # first column of each n-block, then cut the scan
                            if s > 0:
                                tmp = pC.tile([P, NST, 1], F32, tag="ptmp",
                                              name="tmp")
                                nc.vector.tensor_tensor(
                                    tmp[:], dA[:, :, 0:1],
                                    hlast[:, d, :, None], Alu.mult)
                                nc.vector.tensor_tensor(
                                    dBu[:, :, 0:1], dBu[:, :, 0:1], tmp[:],
                                    Alu.add)
                            nc.vector.memset(dA[:, :, 0:1], 0.0)
                            h = pC.tile([P, NST, TSUB], BF16, tag="h",
                                        name="h")
                            nc.vector.tensor_tensor_scan(
                                h[:].rearrange("p n t -> p (n t)"),
                                dA[:].rearrange("p n t -> p (n t)"),
                                dBu[:].rearrange("p n t -> p (n t)"),
                                0.0, Alu.mult, Alu.add)
                            nc.vector.tensor_copy(hlast[:, d, :, None],
                                                  h[:, :, TSUB - 1:TSUB])
                            nc.vector.tensor_tensor(h[:], h[:], c_sub[:],
                                                    Alu.mult)
                            for half_n in (8, 4, 2, 1):
                                nc.vector.tensor_tensor(
                                    h[:, 0:half_n, :], h[:, 0:half_n, :],
                                    h[:, half_n:2 * half_n, :], Alu.add)
                            tg = pC.tile([P, TSUB], BF16, tag="tg", name="tg")
                            nc.vector.scalar_tensor_tensor(
                                tg[:], u_bf[:, d, tsl], dpv[:, d:d + 1],
                                h[:, 0, :], Alu.mult, Alu.add)
                            nc.vector.tensor_tensor(y_g[:, d, tsl], tg[:],
                                                    silz[:, d, tsl], Alu.mult)

            # ---------- Phase 7: out_proj + combine directions ----------
            with tc.tile_pool(name="pE", bufs=1) as pE:
                yp_in = dram.tile([P, KT * L], F32)
                yp_out = dram.tile([P, KT * L], F32)
                ag_out = dram.tile([2, P, KT * L], F32)
                ypv = yp_in[:].rearrange("p (o t) -> p o t", t=L)
                for m in range(KT):
                    pso = [psum.tile([P, 512], F32, tag="mm",
                                     name="pso%d" % _h) for _h in range(2)]
                    for d in range(DT):
                        for h in range(2):
                            nc.tensor.matmul(
                                pso[h][:], outw[:, d, m * P:(m + 1) * P],
                                y_g[:, d, h * 512:(h + 1) * 512],
                                start=(d == 0), stop=(d == DT - 1))
                    ypm = scratch.tile([P, L], F32, tag="big", name="ypm")
                    for h in range(2):
                        nc.vector.tensor_copy(ypm[:, h * 512:(h + 1) * 512],
                                              pso[h][:])
                    nc.sync.dma_start(ypv[:, m, :], ypm[:])
                nc.gpsimd.collective_compute(
                    "AllReduce", Alu.add, replica_groups=PAIRS,
                    ins=[yp_in.opt()], outs=[yp_out.opt()])
                nc.gpsimd.collective_compute(
                    "AllGather", Alu.bypass, replica_groups=XDIR,
                    ins=[yp_out.opt()], outs=[ag_out.opt()])
                ag3 = ag_out[:].rearrange("s p (o t) -> s p o t", t=L)

                # FFN weights (DMA overlaps phase 7 compute)
                w1s = pE.tile([P, KT, DFQ], BF16, tag="w1s")
                nc.sync.dma_start(w1s[:],
                                  w1_t.rearrange("(o p) e -> p o e", p=P))
                w2s = pE.tile([P, DFQ // P, DM], BF16, tag="w2s")
                nc.sync.dma_start(w2s[:],
                                  w2_t.rearrange("(o p) e -> p o e", p=P))

                x2 = pE.tile([P, KT, L], F32, tag="x2")
                xrv = xT_res.rearrange("(o p) t -> p o t", p=P)
                for kt in range(KT):
                    s0 = scratch.tile([P, L], F32, tag="big", name="s0")
                    s1 = scratch.tile([P, L], F32, tag="big", name="s1")
                    xr = scratch.tile([P, L], F32, tag="big", name="xr")
                    nc.sync.dma_start(s0[:], ag3[0, :, kt, :])
                    nc.sync.dma_start(s1[:], ag3[1, :, kt, :])
                    nc.sync.dma_start(xr[:], xrv[:, kt, :])
                    nc.vector.tensor_tensor(s0[:], s0[:], s1[:, ::-1],
                                            Alu.add)
                    nc.vector.scalar_tensor_tensor(
                        x2[:, kt, :], s0[:], 0.5, xr[:], Alu.mult, Alu.add)

                # ---------- Phase 8: LN2 ----------
                mean2 = pE.tile([P, L], F32, tag="mean2")
                rstd2 = pE.tile([P, L], F32, tag="rstd2")
                _ln_stats(nc, pools, lambda kt: x2[:, kt, :], ones_col,
                          rstd2, mean2)
                x2n = pE.tile([P, KT, L], BF16, tag="x2n")
                _ln_apply(nc, pools, lambda kt: x2[:, kt, :], mean2, rstd2,
                          g2, b2v, x2n)

                # ---------- Phase 9: FFN ----------
                h1 = pE.tile([P, DFQ // P, L], BF16, tag="h1")
                for m in range(DFQ // P):
                    psf = [psum.tile([P, 512], F32, tag="mm",
                                     name="psf%d" % _h) for _h in range(2)]
                    for kt in range(KT):
                        for h in range(2):
                            nc.tensor.matmul(
                                psf[h][:], w1s[:, kt, m * P:(m + 1) * P],
                                x2n[:, kt, h * 512:(h + 1) * 512],
                                start=(kt == 0), stop=(kt == KT - 1))
                    for h in range(2):
                        nc.scalar.activation(h1[:, m, h * 512:(h + 1) * 512],
                                             psf[h][:], Act.Gelu,
                                             bias=b1s[:, m:m + 1])
                ff_in = dram.tile([P, KT * L], F32)
                ff_out = dram.tile([P, KT * L], F32)
                ffv = ff_in[:].rearrange("p (o t) -> p o t", t=L)
                for m in range(KT):
                    psg = [psum.tile([P, 512], F32, tag="mm",
                                     name="psg%d" % _h) for _h in range(2)]
                    for kt in range(DFQ // P):
                        for h in range(2):
                            nc.tensor.matmul(
                                psg[h][:], w2s[:, kt, m * P:(m + 1) * P],
                                h1[:, kt, h * 512:(h + 1) * 512],
                                start=(kt == 0), stop=(kt == DFQ // P - 1))
                    ffm = scratch.tile([P, L], F32, tag="big", name="ffm")
                    for h in range(2):
                        nc.vector.tensor_scalar(
                            ffm[:, h * 512:(h + 1) * 512], psg[h][:],
                            b2s[:, m:m + 1], None, Alu.add)
                    nc.sync.dma_start(ffv[:, m, :], ffm[:])
                nc.gpsimd.collective_compute(
                    "AllReduce", Alu.add, replica_groups=QUADS,
                    ins=[ff_in.opt()], outs=[ff_out.opt()])
                ffo = ff_out[:].rearrange("p (o t) -> p o t", t=L)
                outv = outT.rearrange("(o p) t -> p o t", p=P)
                for kt in range(KT):
                    ffs = scratch.tile([P, L], F32, tag="big", name="ffs")
                    nc.sync.dma_start(ffs[:], ffo[:, kt, :])
                    fin = scratch.tile([P, L], F32, tag="big", name="fin")
                    nc.vector.tensor_tensor(fin[:], x2[:, kt, :], ffs[:],
                                            Alu.add)
                    nc.sync.dma_start(outv[:, kt, :], fin[:])

    nc.compile()
    return nc


_NC_CACHE = None


def _get_nc():
    global _NC_CACHE
    if _NC_CACHE is None:
        _NC_CACHE = build_program()
    return _NC_CACHE


def _prep_core(inputs, dir_, b, half):
    hs = slice(half * DIH, (half + 1) * DIH)
    p = "f_" if dir_ == 0 else "b_"
    f32 = np.float32
    xT = np.ascontiguousarray(inputs["x"][b].T.astype(f32))
    m = {}
    m["xT_res"] = xT
    m["xT_ln"] = xT if dir_ == 0 else np.ascontiguousarray(xT[:, ::-1])
    m["ln1_g"] = inputs["norm_g"].astype(f32)
    m["ln1_b"] = inputs["norm_b"].astype(f32)
    m["ln2_g"] = inputs["ffn_g"].astype(f32)
    m["ln2_b"] = inputs["ffn_b"].astype(f32)
    W = inputs[p + "in_proj_w"]
    win = np.concatenate([W[hs], W[2 * DIH + half * DIH:
                                   2 * DIH + (half + 1) * DIH]], axis=0)
    m["win_t"] = np.ascontiguousarray(win.T).astype(_BF)
    m["conv_w"] = np.ascontiguousarray(
        inputs[p + "conv_w"][hs, 0, :]).astype(f32)
    m["conv_b"] = inputs[p + "conv_b"][hs].astype(f32)
    m["a_mat"] = (-np.exp(inputs[p + "A_log"][hs])).astype(f32)
    m["xpw_t"] = np.ascontiguousarray(
        inputs[p + "x_proj_w"][:, hs].T).astype(_BF)
    m["dtw_t"] = np.ascontiguousarray(
        inputs[p + "dt_proj_w"][hs].T).astype(_BF)
    m["dt_b"] = inputs[p + "dt_proj_b"][hs].astype(f32)
    m["d_par"] = inputs[p + "D"][hs].astype(f32)
    m["outw_t"] = np.ascontiguousarray(
        inputs[p + "out_proj_w"][:, hs].T).astype(_BF)
    q = 2 * dir_ + half
    qs = slice(q * DFQ, (q + 1) * DFQ)
    m["w1_t"] = np.ascontiguousarray(inputs["w1"][qs].T).astype(_BF)
    m["b1_q"] = inputs["b1"][qs].astype(f32)
    m["w2_t"] = np.ascontiguousarray(inputs["w2"][:, qs].T).astype(_BF)
    m["b2_e"] = (inputs["b2"] if q == 0
                 else np.zeros_like(inputs["b2"])).astype(f32)
    return m


def make_in_maps(inputs):
    inputs = {k: np.asarray(v) for k, v in inputs.items()}
    maps = []
    for c in range(8):
        dir_, b, half = c // 4, (c % 4) // 2, c % 2
        maps.append(_prep_core(inputs, dir_, b, half))
    return maps


def kernel(**inputs):
    from concourse.bass_utils import run_bass_kernel_spmd
    nc = _get_nc()
    in_maps = make_in_maps(inputs)
    res = run_bass_kernel_spmd(nc, in_maps, core_ids=list(range(8)))
    out0 = res.results[0]["outT"]  # batch 0, [DM, L]
    out1 = res.results[2]["outT"]  # batch 1
    return np.stack([out0.T, out1.T]).astype(np.float32)
